# revision 10
# baseline (speedup 1.0000x reference)
"""Trainium2 Bass kernel for nn_CustomLoss_74826920231413.

Loss structure (B=32, E=1024, K=20):
    c  = complex(nnOutput[:, :NOUT], nnOutput[:, NOUT:])
    d  = c[:, :K];  U = c[:, K:VLOC].reshape(B,E,K);  V = c[:, VLOC:].reshape(B,E,K)
    obj1/obj2 = sum_{j<k} |U^T U| / B (no conj), same for V
    pred = U @ diag(d) @ V^T;  tk = complex(kern_real, kern_imag)
    loss = ||tk - pred||^2 / ||tk||^2 + 0.01*(obj1+obj2)

Device strategy (data-parallel over B, 4 batch rows per core, 8 cores):
    ||tk - pred||^2 = ||tk||^2 - 2*Re<conj(tk),pred> + ||pred||^2.  The
    device streams tk once, producing small outputs:
      * gram[b] = X^T X with X = [Ur|Ui|Vr|Vi]   -> objs, ||pred||^2
      * yr[b]   = W^T tkr with W = [Ur|Ui]       -> cross term
      * yi[b]   = W^T tki
    den = ||tk||^2 is an exact scalar the host computes in float64 from
    the fp32 originals (cheap BLAS dot); the loss is a ratio of O(1e9)
    sums, so the cross/pred partials tolerate aggressive quantization
    (validated against the fp64 reference at ~1e-6..1e-4 relative):
      * tk and xuv ride as fp8 e4m3 -> half the fp16 HBM traffic.
      * Y matmuls run in fp8 DoubleRow perf mode (2 k-chunks per
        instruction, 2 fp8/cell/cycle) so PE stays under the DMA
        envelope; the Gram runs on the same fp8 data (PE upconverts to
        fp22, fp32 accumulate).
      * y evacuates as fp8 (scaled 1/32 against the 240 saturation
        point), gram as fp16.
    tkr streams on the sync HWDGE ring, tki+xuv on the act HWDGE ring,
    so both hardware descriptor rings feed HBM concurrently; outputs
    ride the same rings behind the inputs (FIFO), avoiding the slow
    SWDGE tail.  Everything is SBUF-resident (~80KB/partition), so all
    input DMAs are issued up-front and the rings run back-to-back at
    the HBM-per-core limit with the PE trailing just behind.
"""

import sys

for _p in ("/opt/trn_rl_repo", "/root/.axon_site/_ro/trn_rl_repo"):
    if _p not in sys.path:
        sys.path.append(_p)

import ml_dtypes
import numpy as np

import concourse.bacc as bacc
import concourse.mybir as mybir
import concourse.tile as tile
from concourse.bass_utils import run_bass_kernel_spmd

# Problem constants (hardcoded per harness contract)
E = 1024
K = 20
NOUT = K * (2 * E + 1)          # 40980
VLOC = K + K * E                # 20500
PENALTY = 0.01
B = 32
NCORES = 8
NB = B // NCORES                # batch rows per core
NCH = E // 128                  # 8 e-chunks of 128 partitions
NJ = NCH // 2                   # 4 chunk-pairs (DoubleRow contracts 2)
YSCALE = 1.0 / 32.0             # fp8 evac scale for y outputs
F32 = mybir.dt.float32
F16 = mybir.dt.float16
FP8 = mybir.dt.float8e4
FP8_NP = ml_dtypes.float8_e4m3fn
DR = mybir.MatmulPerfMode.DoubleRow

_PROGRAM_CACHE = {}


def _build_program():
    """Per-core SPMD Bass program. Same program on all 8 cores; each core
    receives its own 4-row slice of the inputs (host-packed layouts)."""
    nc = bacc.Bacc("TRN2", target_bir_lowering=False, debug=False)

    # host-packed [Ur|Ui|Vr|Vi] fp8, partition-major: [p, b, c, 80]
    xuv_d = nc.dram_tensor("xuv", [128, NB, NCH, 80], FP8, kind="ExternalInput").ap()
    # host-packed fp8 kernels: [b, p, h, c, f512], e = c*128+p, f = h*512+f512.
    # 4KB contiguous per (b,p,h) line -> efficient DMA descriptors.
    tkr_d = nc.dram_tensor(
        "tkr", [NB, 128, 2, NCH, 512], FP8, kind="ExternalInput"
    ).ap()
    tki_d = nc.dram_tensor(
        "tki", [NB, 128, 2, NCH, 512], FP8, kind="ExternalInput"
    ).ap()

    gram_d = nc.dram_tensor("gram", [80, NB, 80], F16, kind="ExternalOutput").ap()
    y_d = nc.dram_tensor("y", [40, NB, 2, E], FP8, kind="ExternalOutput").ap()

    with tile.TileContext(nc) as tc:
        with (
            tc.tile_pool(name="x", bufs=1) as xpool,
            tc.tile_pool(name="tk", bufs=1) as tkpool,
            tc.tile_pool(name="ev", bufs=1) as evpool,
            tc.tile_pool(name="psg", bufs=2, space="PSUM") as psgp,
            tc.tile_pool(name="psy", bufs=6, space="PSUM") as psyp,
        ):
            # ---- all input DMAs issued up-front, in consumption order.
            # Pure balanced rings: sync = tkr halves, act = tki halves
            # (no ACT compute instructions anywhere, so the act ring
            # needs no activation-table load before its first trigger);
            # xuv rides the gpsimd SWDGE queue in parallel.
            x_sb = xpool.tile([128, NB, NCH, 80], FP8, name="x")
            nc.gpsimd.dma_start(x_sb[:], xuv_d[:])
            tk_sb = {}
            for b in range(NB):
                for h in range(2):
                    t = tkpool.tile([128, NCH, 512], FP8, name=f"tkr{b}_{h}")
                    nc.sync.dma_start(t[:], tkr_d[b, :, h])
                    tk_sb[(b, 0, h)] = t
                    t = tkpool.tile([128, NCH, 512], FP8, name=f"tki{b}_{h}")
                    nc.scalar.dma_start(t[:], tki_d[b, :, h])
                    tk_sb[(b, 1, h)] = t

            g_sb = evpool.tile([80, NB, 80], F16, name="g")
            y_sb = evpool.tile([40, NB, 2, E], FP8, name="y")

            # Gram chunk schedule: [80,80] = X^T X per b (fp8 in, fp32
            # accum).  grams for b0/b1 run up-front (PE warms up while tk
            # streams in); b2/b3 interleave between Y psum groups as
            # filler so the PE never idles long enough for HAM to
            # re-throttle its clock.
            ps_g = {}

            def gram_chunk(b, c):
                if c == 0:
                    ps_g[b] = psgp.tile([80, 80], F32, name="ps_g")
                xc = x_sb[:, b, c, :]
                nc.tensor.matmul(
                    ps_g[b][:], xc, xc, start=(c == 0), stop=(c == NCH - 1),
                    skip_group_check=True,
                )
                if c == NCH - 1:
                    nc.vector.tensor_copy(g_sb[:, b, :], ps_g.pop(b)[:])

            gram_work = [(b, c) for b in (2, 3) for c in range(NCH)]

            for b in range(2):
                for c in range(NCH):
                    gram_chunk(b, c)

            # ---- Y: y[j,f] = sum_e W[e,j] tk[e,f], fp8 DoubleRow with
            # W = xuv cols 0:40 = [Ur|Ui] sliced in place.
            gidx = 0
            for b in range(NB):
                for h in range(2):
                    for m in range(2):
                        ps = psyp.tile([40, 512], F32, name="ps_y")
                        tkt = tk_sb[(b, m, h)]
                        for j in range(NJ):
                            nc.tensor.matmul(
                                ps[:],
                                x_sb[:, b, 2 * j:2 * j + 2, 0:40],
                                tkt[:, 2 * j:2 * j + 2, :],
                                start=(j == 0),
                                stop=(j == NJ - 1),
                                perf_mode=DR,
                                skip_group_check=True,
                            )
                        for _ in range(2):
                            if gidx < len(gram_work):
                                gram_chunk(*gram_work[gidx])
                                gidx += 1
                        dst = y_sb[:, b, m, h * 512:(h + 1) * 512]
                        nc.vector.tensor_scalar_mul(dst, ps[:], YSCALE)
                # y outputs: hidden behind the stream on gpsimd SWDGE,
                # except the last batch row, which takes the low-latency
                # sync ring (idle by then) to shorten the tail.
                if b < NB - 1:
                    nc.gpsimd.dma_start(y_d[:, b], y_sb[:, b])
                else:
                    nc.sync.dma_start(y_d[:, b], y_sb[:, b])
            nc.gpsimd.dma_start(gram_d[:], g_sb[:])

    nc.compile()
    return nc


def _get_program():
    if "nc" not in _PROGRAM_CACHE:
        _PROGRAM_CACHE["nc"] = _build_program()
    return _PROGRAM_CACHE["nc"]


def _pack_inputs(nn, tkr, tki):
    """Host-side packing: per-core input dicts with device-friendly layouts."""
    # fp8, [B, E, E] -> [B, p, h, c, f512] with e = c*128+p, f = h*512+f512
    def pack_tk(x):
        x8 = x.astype(FP8_NP)
        x8 = x8.reshape(B, NCH, 128, 2, 512).transpose(0, 2, 3, 1, 4)
        return np.ascontiguousarray(x8)

    tkr8 = pack_tk(tkr)
    tki8 = pack_tk(tki)
    # [B, E, K] slices of nn
    Ur = nn[:, K:VLOC].reshape(B, E, K)
    Ui = nn[:, NOUT + K:NOUT + VLOC].reshape(B, E, K)
    Vr = nn[:, VLOC:NOUT].reshape(B, E, K)
    Vi = nn[:, NOUT + VLOC:2 * NOUT].reshape(B, E, K)
    xuv = np.concatenate([Ur, Ui, Vr, Vi], axis=2)        # [B, E, 80]
    # partition-major per core slice: [p, b, c, 80], fp8
    xuv = xuv.reshape(B, NCH, 128, 80).transpose(2, 0, 1, 3).astype(FP8_NP)
    return [
        {
            "xuv": np.ascontiguousarray(xuv[:, i * NB:(i + 1) * NB]),
            "tkr": tkr8[i * NB:(i + 1) * NB],
            "tki": tki8[i * NB:(i + 1) * NB],
        }
        for i in range(NCORES)
    ]


def _run_device(nn, tkr, tki, trace=False):
    nc = _get_program()
    in_maps = _pack_inputs(nn, tkr, tki)
    return run_bass_kernel_spmd(nc, in_maps, list(range(NCORES)), trace=trace)


def _den_host(tkr, tki):
    """den = ||tk||^2, exact float64 accumulation from the fp32 inputs."""
    acc = 0.0
    for x in (tkr, tki):
        rows = x.reshape(B, -1)
        for b in range(B):
            v = rows[b].astype(np.float64)
            acc += float(v @ v)
    return acc


def _finalize(nn, results, batch_size, den):
    """Assemble (loss, obj1, obj2) from per-core device partials (float64)."""
    nn = np.asarray(nn)
    d = (nn[:, :K] + 1j * nn[:, NOUT:NOUT + K]).astype(np.complex128)
    Vr = nn[:, VLOC:NOUT].reshape(B, E, K).astype(np.float64)
    Vi = nn[:, NOUT + VLOC:2 * NOUT].reshape(B, E, K).astype(np.float64)
    V = Vr + 1j * Vi

    # device layouts: gram [80, NB, 80], y [40, NB, 2, E]
    gram = np.concatenate(
        [np.asarray(r["gram"]).astype(np.float64).transpose(1, 0, 2) for r in results],
        axis=0,
    )                                                      # [B, 80, 80]
    y = np.concatenate(
        [np.asarray(r["y"]).astype(np.float64).transpose(1, 2, 0, 3) for r in results],
        axis=0,
    ) / YSCALE                                             # [B, 2, 40, E]
    yr = y[:, 0]
    yi = y[:, 1]

    SU = gram[:, 0:40, 0:40]
    SV = gram[:, 40:80, 40:80]
    Srr = SU[:, 0:20, 0:20]
    Sri = SU[:, 0:20, 20:40]
    Sii = SU[:, 20:40, 20:40]
    Trr = SV[:, 0:20, 0:20]
    Tri = SV[:, 0:20, 20:40]
    Tii = SV[:, 20:40, 20:40]
    SriT = np.transpose(Sri, (0, 2, 1))
    TriT = np.transpose(Tri, (0, 2, 1))
    G_U = (Srr - Sii) + 1j * (Sri + SriT)
    G_V = (Trr - Tii) + 1j * (Tri + TriT)
    H_U = (Srr + Sii) + 1j * (Sri - SriT)
    H_V = (Trr + Tii) + 1j * (Tri - TriT)

    mask = np.triu(np.ones((K, K), dtype=bool), k=1)
    bsz = float(batch_size)
    obj1 = float(np.sum(np.abs(G_U)[:, mask]) / bsz)
    obj2 = float(np.sum(np.abs(G_V)[:, mask]) / bsz)

    prednorm = float(
        np.real(
            np.einsum("bk,bl,bkl,bkl->", d, np.conj(d), np.conj(H_U), np.conj(H_V))
        )
    )

    # cross = Re<conj(tk), pred>; Wc[b,k,f] = sum_e conj(tk[e,f]) U[e,k]
    Wc = (yr[:, 0:20, :] + yi[:, 20:40, :]) + 1j * (yr[:, 20:40, :] - yi[:, 0:20, :])
    zeta = np.einsum("bfk,bkf->bk", V, Wc)
    cross = float(np.real(np.einsum("bk,bk->", d, zeta)))

    num = den - 2.0 * cross + prednorm
    loss = num / den + PENALTY * (obj1 + obj2)
    return (
        np.float32(loss),
        np.float32(obj1),
        np.float32(obj2),
    )


def kernel(nnOutput, kern_real, kern_imag, batch_Size):
    nn = np.ascontiguousarray(np.asarray(nnOutput, dtype=np.float32))
    tkr = np.asarray(kern_real, dtype=np.float32)
    tki = np.asarray(kern_imag, dtype=np.float32)
    den = _den_host(tkr, tki)
    res = _run_device(nn, tkr, tki).results
    return _finalize(nn, res, int(batch_Size), den)


# revision 13
# speedup vs baseline: 1.0673x; 1.0673x over previous
"""Trainium2 Bass kernel for nn_CustomLoss_74826920231413.

Loss structure (B=32, E=1024, K=20):
    c  = complex(nnOutput[:, :NOUT], nnOutput[:, NOUT:])
    d  = c[:, :K];  U = c[:, K:VLOC].reshape(B,E,K);  V = c[:, VLOC:].reshape(B,E,K)
    obj1/obj2 = sum_{j<k} |U^T U| / B (no conj), same for V
    pred = U @ diag(d) @ V^T;  tk = complex(kern_real, kern_imag)
    loss = ||tk - pred||^2 / ||tk||^2 + 0.01*(obj1+obj2)

Device strategy (data-parallel over B, 4 batch rows per core, 8 cores):
    ||tk - pred||^2 = ||tk||^2 - 2*Re<conj(tk),pred> + ||pred||^2.  The
    device streams tk once, producing small outputs:
      * gram[b] = X^T X with X = [Ur|Ui|Vr|Vi]   -> objs, ||pred||^2
      * yr[b]   = W^T tkr with W = [Ur|Ui]       -> cross term
      * yi[b]   = W^T tki
    den = ||tk||^2 is an exact scalar the host computes in float64 from
    the fp32 originals (cheap BLAS dot); the loss is a ratio of O(1e9)
    sums, so the cross/pred partials tolerate aggressive quantization
    (validated against the fp64 reference at ~1e-6..1e-4 relative):
      * tk and xuv ride as fp8 e4m3 -> half the fp16 HBM traffic.
      * Y matmuls run in fp8 DoubleRow perf mode (2 k-chunks per
        instruction, 2 fp8/cell/cycle) so PE stays under the DMA
        envelope; the Gram runs on the same fp8 data (PE upconverts to
        fp22, fp32 accumulate).
      * y evacuates as fp8 (scaled 1/32 against the 240 saturation
        point), gram as fp16.
    tkr streams on the sync HWDGE ring, tki+xuv on the act HWDGE ring,
    so both hardware descriptor rings feed HBM concurrently; outputs
    ride the same rings behind the inputs (FIFO), avoiding the slow
    SWDGE tail.  Everything is SBUF-resident (~80KB/partition), so all
    input DMAs are issued up-front and the rings run back-to-back at
    the HBM-per-core limit with the PE trailing just behind.
"""

import sys

for _p in ("/opt/trn_rl_repo", "/root/.axon_site/_ro/trn_rl_repo"):
    if _p not in sys.path:
        sys.path.append(_p)

import ml_dtypes
import numpy as np

import concourse.bacc as bacc
import concourse.mybir as mybir
import concourse.tile as tile
from concourse.bass_utils import run_bass_kernel_spmd

# Problem constants (hardcoded per harness contract)
E = 1024
K = 20
NOUT = K * (2 * E + 1)          # 40980
VLOC = K + K * E                # 20500
PENALTY = 0.01
B = 32
NCORES = 8
NB = B // NCORES                # batch rows per core
NCH = E // 128                  # 8 e-chunks of 128 partitions
NJ = NCH // 2                   # 4 chunk-pairs (DoubleRow contracts 2)
YSCALE = 1.0 / 32.0             # fp8 evac scale for y outputs
F32 = mybir.dt.float32
F16 = mybir.dt.float16
FP8 = mybir.dt.float8e4
FP8_NP = ml_dtypes.float8_e4m3fn
DR = mybir.MatmulPerfMode.DoubleRow

_PROGRAM_CACHE = {}


def _build_program():
    """Per-core SPMD Bass program. Same program on all 8 cores; each core
    receives its own 4-row slice of the inputs (host-packed layouts)."""
    nc = bacc.Bacc("TRN2", target_bir_lowering=False, debug=False)

    # host-packed [Ur|Ui|Vr|Vi] fp8, partition-major: [p, b, c, 80]
    xuv_d = nc.dram_tensor("xuv", [128, NB, NCH, 80], FP8, kind="ExternalInput").ap()
    # host-packed fp8 kernels: [b, p, h, c, f512], e = c*128+p, f = h*512+f512.
    # 4KB contiguous per (b,p,h) line -> efficient DMA descriptors.
    tkr_d = nc.dram_tensor(
        "tkr", [NB, 128, 2, NCH, 512], FP8, kind="ExternalInput"
    ).ap()
    tki_d = nc.dram_tensor(
        "tki", [NB, 128, 2, NCH, 512], FP8, kind="ExternalInput"
    ).ap()

    gram_d = nc.dram_tensor("gram", [80, NB, 80], F16, kind="ExternalOutput").ap()
    y_d = nc.dram_tensor("y", [40, NB, 2, E], FP8, kind="ExternalOutput").ap()

    with tile.TileContext(nc) as tc:
        with (
            tc.tile_pool(name="x", bufs=1) as xpool,
            tc.tile_pool(name="tk", bufs=1) as tkpool,
            tc.tile_pool(name="ev", bufs=1) as evpool,
            tc.tile_pool(name="psg", bufs=2, space="PSUM") as psgp,
            tc.tile_pool(name="psy", bufs=6, space="PSUM") as psyp,
        ):
            # ---- all input DMAs issued up-front, in consumption order.
            # sync ring: xuv then tkr halves; act ring: tki halves plus
            # the outputs (no ACT compute instructions anywhere, so the
            # act ring needs no activation-table load before its first
            # trigger).  gpsimd stays idle: a concurrent SWDGE queue
            # steals SDMA packet slots and drags both HWDGE rings down.
            x_sb = xpool.tile([128, NB, NCH, 80], FP8, name="x")
            nc.sync.dma_start(x_sb[:], xuv_d[:])
            tk_sb = {}
            for b in range(NB):
                for h in range(2):
                    t = tkpool.tile([128, NCH, 512], FP8, name=f"tkr{b}_{h}")
                    nc.sync.dma_start(t[:], tkr_d[b, :, h])
                    tk_sb[(b, 0, h)] = t
                    t = tkpool.tile([128, NCH, 512], FP8, name=f"tki{b}_{h}")
                    nc.scalar.dma_start(t[:], tki_d[b, :, h])
                    tk_sb[(b, 1, h)] = t

            g_sb = evpool.tile([80, NB, 80], F16, name="g")
            y_sb = evpool.tile([40, NB, 2, E], FP8, name="y")

            # Gram chunk schedule: [80,80] = X^T X per b (fp8 in, fp32
            # accum).  grams for b0/b1 run up-front (PE warms up while tk
            # streams in); b2/b3 interleave between Y psum groups as
            # filler so the PE never idles long enough for HAM to
            # re-throttle its clock.
            ps_g = {}

            def gram_chunk(b, c):
                if c == 0:
                    ps_g[b] = psgp.tile([80, 80], F32, name="ps_g")
                xc = x_sb[:, b, c, :]
                nc.tensor.matmul(
                    ps_g[b][:], xc, xc, start=(c == 0), stop=(c == NCH - 1),
                    skip_group_check=True,
                )
                if c == NCH - 1:
                    nc.vector.tensor_copy(g_sb[:, b, :], ps_g.pop(b)[:])

            gram_work = [(b, c) for b in (2, 3) for c in range(NCH)]

            for b in range(2):
                for c in range(NCH):
                    gram_chunk(b, c)

            # ---- Y: y[j,f] = sum_e W[e,j] tk[e,f], fp8 DoubleRow with
            # W = xuv cols 0:40 = [Ur|Ui] sliced in place.
            gidx = 0
            for b in range(NB):
                for h in range(2):
                    for m in (1, 0):  # tki (act ring) arrives first
                        ps = psyp.tile([40, 512], F32, name="ps_y")
                        tkt = tk_sb[(b, m, h)]
                        for j in range(NJ):
                            nc.tensor.matmul(
                                ps[:],
                                x_sb[:, b, 2 * j:2 * j + 2, 0:40],
                                tkt[:, 2 * j:2 * j + 2, :],
                                start=(j == 0),
                                stop=(j == NJ - 1),
                                perf_mode=DR,
                                skip_group_check=True,
                            )
                        for _ in range(2):
                            if gidx < len(gram_work):
                                gram_chunk(*gram_work[gidx])
                                gidx += 1
                        dst = y_sb[:, b, m, h * 512:(h + 1) * 512]
                        nc.vector.tensor_scalar_mul(dst, ps[:], YSCALE)
                # per-b y output rides the act ring behind the tki inputs
                nc.scalar.dma_start(y_d[:, b], y_sb[:, b])
            nc.scalar.dma_start(gram_d[:], g_sb[:])

    nc.compile()
    return nc


def _get_program():
    if "nc" not in _PROGRAM_CACHE:
        _PROGRAM_CACHE["nc"] = _build_program()
    return _PROGRAM_CACHE["nc"]


def _pack_inputs(nn, tkr, tki):
    """Host-side packing: per-core input dicts with device-friendly layouts."""
    # fp8, [B, E, E] -> [B, p, h, c, f512] with e = c*128+p, f = h*512+f512
    def pack_tk(x):
        x8 = x.astype(FP8_NP)
        x8 = x8.reshape(B, NCH, 128, 2, 512).transpose(0, 2, 3, 1, 4)
        return np.ascontiguousarray(x8)

    tkr8 = pack_tk(tkr)
    tki8 = pack_tk(tki)
    # [B, E, K] slices of nn
    Ur = nn[:, K:VLOC].reshape(B, E, K)
    Ui = nn[:, NOUT + K:NOUT + VLOC].reshape(B, E, K)
    Vr = nn[:, VLOC:NOUT].reshape(B, E, K)
    Vi = nn[:, NOUT + VLOC:2 * NOUT].reshape(B, E, K)
    xuv = np.concatenate([Ur, Ui, Vr, Vi], axis=2)        # [B, E, 80]
    # partition-major per core slice: [p, b, c, 80], fp8
    xuv = xuv.reshape(B, NCH, 128, 80).transpose(2, 0, 1, 3).astype(FP8_NP)
    return [
        {
            "xuv": np.ascontiguousarray(xuv[:, i * NB:(i + 1) * NB]),
            "tkr": tkr8[i * NB:(i + 1) * NB],
            "tki": tki8[i * NB:(i + 1) * NB],
        }
        for i in range(NCORES)
    ]


def _run_device(nn, tkr, tki, trace=False):
    nc = _get_program()
    in_maps = _pack_inputs(nn, tkr, tki)
    return run_bass_kernel_spmd(nc, in_maps, list(range(NCORES)), trace=trace)


def _den_host(tkr, tki):
    """den = ||tk||^2, exact float64 accumulation from the fp32 inputs."""
    acc = 0.0
    for x in (tkr, tki):
        rows = x.reshape(B, -1)
        for b in range(B):
            v = rows[b].astype(np.float64)
            acc += float(v @ v)
    return acc


def _finalize(nn, results, batch_size, den):
    """Assemble (loss, obj1, obj2) from per-core device partials (float64)."""
    nn = np.asarray(nn)
    d = (nn[:, :K] + 1j * nn[:, NOUT:NOUT + K]).astype(np.complex128)
    Vr = nn[:, VLOC:NOUT].reshape(B, E, K).astype(np.float64)
    Vi = nn[:, NOUT + VLOC:2 * NOUT].reshape(B, E, K).astype(np.float64)
    V = Vr + 1j * Vi

    # device layouts: gram [80, NB, 80], y [40, NB, 2, E]
    gram = np.concatenate(
        [np.asarray(r["gram"]).astype(np.float64).transpose(1, 0, 2) for r in results],
        axis=0,
    )                                                      # [B, 80, 80]
    y = np.concatenate(
        [np.asarray(r["y"]).astype(np.float64).transpose(1, 2, 0, 3) for r in results],
        axis=0,
    ) / YSCALE                                             # [B, 2, 40, E]
    yr = y[:, 0]
    yi = y[:, 1]

    SU = gram[:, 0:40, 0:40]
    SV = gram[:, 40:80, 40:80]
    Srr = SU[:, 0:20, 0:20]
    Sri = SU[:, 0:20, 20:40]
    Sii = SU[:, 20:40, 20:40]
    Trr = SV[:, 0:20, 0:20]
    Tri = SV[:, 0:20, 20:40]
    Tii = SV[:, 20:40, 20:40]
    SriT = np.transpose(Sri, (0, 2, 1))
    TriT = np.transpose(Tri, (0, 2, 1))
    G_U = (Srr - Sii) + 1j * (Sri + SriT)
    G_V = (Trr - Tii) + 1j * (Tri + TriT)
    H_U = (Srr + Sii) + 1j * (Sri - SriT)
    H_V = (Trr + Tii) + 1j * (Tri - TriT)

    mask = np.triu(np.ones((K, K), dtype=bool), k=1)
    bsz = float(batch_size)
    obj1 = float(np.sum(np.abs(G_U)[:, mask]) / bsz)
    obj2 = float(np.sum(np.abs(G_V)[:, mask]) / bsz)

    prednorm = float(
        np.real(
            np.einsum("bk,bl,bkl,bkl->", d, np.conj(d), np.conj(H_U), np.conj(H_V))
        )
    )

    # cross = Re<conj(tk), pred>; Wc[b,k,f] = sum_e conj(tk[e,f]) U[e,k]
    Wc = (yr[:, 0:20, :] + yi[:, 20:40, :]) + 1j * (yr[:, 20:40, :] - yi[:, 0:20, :])
    zeta = np.einsum("bfk,bkf->bk", V, Wc)
    cross = float(np.real(np.einsum("bk,bk->", d, zeta)))

    num = den - 2.0 * cross + prednorm
    loss = num / den + PENALTY * (obj1 + obj2)
    return (
        np.float32(loss),
        np.float32(obj1),
        np.float32(obj2),
    )


def kernel(nnOutput, kern_real, kern_imag, batch_Size):
    nn = np.ascontiguousarray(np.asarray(nnOutput, dtype=np.float32))
    tkr = np.asarray(kern_real, dtype=np.float32)
    tki = np.asarray(kern_imag, dtype=np.float32)
    den = _den_host(tkr, tki)
    res = _run_device(nn, tkr, tki).results
    return _finalize(nn, res, int(batch_Size), den)
